# revision 11
# baseline (speedup 1.0000x reference)
"""GNN message passing + 3x conv3x3 + leaky-relu, distributed over 8 trn2 NeuronCores.

Strategy (node-sharded, 128 nodes/core):
- Pooling (pos/neg masked bidirectional scatter-add) is a dense matmul:
  pooled_s[own 128 nodes, C*HW] = sum_k A_s^T[chunk_k, own].T @ F[chunk_k, C*HW]
  with A_s the [N, N] signed-adjacency count matrix (small ints, exact in
  bf16) built on host from the edge list. Full 128x128 PE utilization,
  F (feats, bf16) streamed once from HBM. No edge-dependent program.
- Pooled rows are reshaped into conv layout (4 nodes x (16 pos + 16 neg)
  partitions) by SBUF->SBUF DMAs, then copied into 34x34 zero-padded grids.
- Convs run as 9 shifted-tap matmuls over the padded grid using strided
  access-pattern views (no im2col), bf16 operands, fp32 PSUM accumulation.
  Matmul issue order is tap-outer / (bundle, rowgroup)-inner so all 16
  32x32 tile_position sub-arrays stream concurrently (MATMUL starts are
  pc-monotone; same-position chains serialize, so interleave across
  positions).
- leaky_relu(x) = x + relu(-0.9 x): one ScalarE activation + one VectorE
  tensor_tensor add per bundle, full 128-lane ops.
"""

import numpy as np

N, C, H, W = 1024, 16, 32, 32
NCORES = 8
NPC = N // NCORES            # nodes per core
CONV_ROUNDS = NPC // 16      # 16 nodes per conv round (4 bundles)
HP = WP = H + 2
GRID = HP * WP
HW = H * W
CHW = C * HW                 # 16384
KCH = N // 128               # 8 K-chunks for the pooling matmul
FT = CHW // 512              # 32 free-dim tiles of 512 for the pooling matmul

_prog_cache = {}


def _make_tile_context(nc):
    """TileContext whose lowering splits multi-sem waits onto nop carriers
    (this walrus build accepts at most one sync wait per instruction) and
    whose tail drain does the same."""
    import concourse.mybir as mybir
    import concourse.tile as tile

    class _TC(tile.TileContext):
        def _lower_ordered_insts(self, ordered):
            for bb_name, insts in ordered.items():
                out = []
                for inst in insts:
                    si = inst.sync_info
                    waits = list(si.on_wait) if si is not None and si.on_wait else []
                    if len(waits) > 1:
                        for w in waits[:-1]:
                            car = mybir.InstNoOp(
                                name=self.nc.get_next_instruction_name(),
                                ins=[], outs=[])
                            car.engine = inst.engine
                            car.sync_info = mybir.SyncInfo(on_wait=[w], on_update=[])
                            self.nc.register_instruction(car, overwrite=True)
                            out.append(car)
                        inst.sync_info = mybir.SyncInfo(
                            on_wait=[waits[-1]],
                            on_update=list(si.on_update) if si.on_update else [])
                    out.append(inst)
                insts[:] = out
            return super()._lower_ordered_insts(ordered)

        def _drain_and_barrier(self, tick_clock, wait_clock):
            clock = tick_clock.global_clock
            allocated = wait_clock.sems.allocated()
            for proc, tick in enumerate(clock):
                if tick > 0 and proc in allocated:
                    n = self.nc.sync.nop(nofuse=True, hint="tailwait")
                    n.wait_op(allocated[proc], tick, "sem-ge")
            self.nc.sync.drain()
            self.nc.all_engine_barrier()
            assert self.sems is not None
            popped = self.nc._tile_sem_poison_stack.pop()
            assert popped is self._sem_poison
            self.nc.clear_and_free_semaphores(list(self.sems.allocated().values()))
            self.nc.all_engine_barrier()

    return _TC(nc)


def _build_program(variant="full"):
    import concourse.bass as bass
    import concourse.mybir as mybir

    do_pool = variant in ("full", "pool")
    do_conv = variant in ("full", "conv")

    f32 = mybir.dt.float32
    bf16 = mybir.dt.bfloat16
    AF = mybir.ActivationFunctionType
    ALU = mybir.AluOpType

    nc = bass.Bass()
    tabbf_d = nc.dram_tensor("tabbf", [N, CHW], bf16, kind="ExternalInput")
    apos_d = nc.dram_tensor("apos", [N, NPC], bf16, kind="ExternalInput")
    aneg_d = nc.dram_tensor("aneg", [N, NPC], bf16, kind="ExternalInput")
    fown_d = nc.dram_tensor("fown", [NPC * C, HW], bf16, kind="ExternalInput")
    w1pn_d = nc.dram_tensor("w1pn", [128, 9 * 32], bf16, kind="ExternalInput")
    w1s_d = nc.dram_tensor("w1s", [128, 9 * 32], bf16, kind="ExternalInput")
    w2_d = nc.dram_tensor("w2", [128, 9 * 32], bf16, kind="ExternalInput")
    w3_d = nc.dram_tensor("w3", [128, 9 * 16], bf16, kind="ExternalInput")
    y_d = nc.dram_tensor("y", [NPC * C, HW], f32, kind="ExternalOutput")
    # DRAM scratch for pooled results, row n = own node, [C, HW] per row
    scr_pos_d = nc.dram_tensor("scr_pos", [NPC, CHW], bf16, kind="Internal")
    scr_neg_d = nc.dram_tensor("scr_neg", [NPC, CHW], bf16, kind="Internal")

    def valid(ap_grid):
        # [p, GRID] tile AP -> [p, 32, 32] interior view of the 34x34 grid
        return ap_grid.rearrange("p (h w) -> p h w", w=WP)[:, 1:H + 1, 1:W + 1]

    def tap_view(ap_grid, base, k, dy, dx, h0):
        # rhs view for tap (dy,dx), output rows [h0, h0+16), K channels at
        # partition `base`
        g3 = ap_grid.rearrange("p (h w) -> p h w", w=WP)
        return g3[base:base + k, h0 + dy:h0 + dy + 16, dx:dx + W]

    tc = _make_tile_context(nc)
    with tc:
        with (tc.tile_pool(name="cw", bufs=1) as cw,
              tc.tile_pool(name="rhsp", bufs=3) as rhsp,
              tc.tile_pool(name="poolbuf", bufs=3) as poolbuf,
              tc.tile_pool(name="ppp", bufs=4) as ppp,
              tc.tile_pool(name="fop", bufs=8) as fop,
              tc.tile_pool(name="x1pnp", bufs=8) as x1pnp,
              tc.tile_pool(name="x1sp", bufs=8) as x1sp,
              tc.tile_pool(name="x2p", bufs=4) as x2p,
              tc.tile_pool(name="x3p", bufs=4) as x3p,
              tc.tile_pool(name="r2p", bufs=4) as r2p,
              tc.tile_pool(name="osbp", bufs=4) as osbp,
              tc.tile_pool(name="psp", bufs=4, space="PSUM") as psp):
            # ---- constant loads
            apos_t = cw.tile([128, N], bf16)
            aneg_t = cw.tile([128, N], bf16)
            for k in range(KCH):
                nc.sync.dma_start(out=apos_t[:, 128 * k:128 * (k + 1)],
                                  in_=apos_d[128 * k:128 * (k + 1), :])
                nc.sync.dma_start(out=aneg_t[:, 128 * k:128 * (k + 1)],
                                  in_=aneg_d[128 * k:128 * (k + 1), :])
            w1pn_t = cw.tile([128, 9 * 32], bf16)
            nc.sync.dma_start(out=w1pn_t[:], in_=w1pn_d[:])
            w1s_t = cw.tile([128, 9 * 32], bf16)
            nc.sync.dma_start(out=w1s_t[:], in_=w1s_d[:])
            w2_t = cw.tile([128, 9 * 32], bf16)
            nc.sync.dma_start(out=w2_t[:], in_=w2_d[:])
            w3_t = cw.tile([128, 9 * 16], bf16)
            nc.sync.dma_start(out=w3_t[:], in_=w3_d[:])

            # ---- pooling: pooled_s[own n, CHW] = A_s^T.T @ F, bf16->f32 psum
            # Streamed per 512-col free tile to DRAM scratch (row n holds
            # node n's [C, HW] block; scr col = c*HW + hw).
            if do_pool:
                for ft in range(FT):
                    rhs_t = rhsp.tile([128, KCH * 512], bf16, tag="rhs")
                    nc.sync.dma_start(
                        out=rhs_t[:].rearrange("p (k f) -> p k f", k=KCH),
                        in_=tabbf_d[:, 512 * ft:512 * (ft + 1)]
                            .rearrange("(k p) f -> p k f", p=128))
                    ps = psp.tile([128, HW], f32, tag="ps")
                    for k in range(KCH):
                        nc.tensor.matmul(
                            out=ps[:, 0:512],
                            lhsT=apos_t[:, 128 * k:128 * (k + 1)],
                            rhs=rhs_t[:, 512 * k:512 * (k + 1)],
                            start=(k == 0), stop=(k == KCH - 1))
                    for k in range(KCH):
                        nc.tensor.matmul(
                            out=ps[:, 512:1024],
                            lhsT=aneg_t[:, 128 * k:128 * (k + 1)],
                            rhs=rhs_t[:, 512 * k:512 * (k + 1)],
                            start=(k == 0), stop=(k == KCH - 1))
                    stg = poolbuf.tile([128, HW], bf16, tag="stg")
                    nc.vector.tensor_copy(out=stg[:], in_=ps[:])
                    nc.sync.dma_start(
                        out=scr_pos_d[:, 512 * ft:512 * (ft + 1)],
                        in_=stg[:, 0:512])
                    nc.sync.dma_start(
                        out=scr_neg_d[:, 512 * ft:512 * (ft + 1)],
                        in_=stg[:, 512:1024])
            else:
                zstg = poolbuf.tile([128, HW], bf16, tag="zstg")
                nc.vector.memset(zstg[:], 0.0)
                for si_d in (scr_pos_d, scr_neg_d):
                    for i in range(CHW // HW):
                        nc.sync.dma_start(
                            out=si_d[:, HW * i:HW * (i + 1)], in_=zstg[:])

            if not do_conv:
                # park something in y so the output is defined
                zz = osbp.tile([128, HW], f32, tag="osb")
                nc.vector.memset(zz[:], 0.0)
                for slot in range(NPC):
                    nc.sync.dma_start(out=y_d[C * slot:C * slot + C, :],
                                      in_=zz[0:C, :])
                return nc

            memset_count = {}

            def fresh(pool, name, width, bufs):
                t = pool.tile([128, width], bf16, tag=name)
                c = memset_count.get(name, 0)
                if c < bufs:
                    nc.vector.memset(t[:], 0.0)
                    memset_count[name] = c + 1
                return t

            for rnd in range(CONV_ROUNDS):
                # ---- build grids for the 4 bundles of this round
                x1_t, x1s_t = [], []
                for b in range(4):
                    s0 = 16 * rnd + 4 * b
                    pp = ppp.tile([128, HW], bf16, tag="pp")
                    for si, scr in ((0, scr_pos_d), (1, scr_neg_d)):
                        for j in range(4):
                            nc.sync.dma_start(
                                out=pp[32 * j + 16 * si:
                                       32 * j + 16 * si + C, :],
                                in_=scr[s0 + j, :]
                                    .rearrange("(c f) -> c f", c=C))
                    x1 = fresh(x1pnp, "x1pn", GRID, 8)
                    nc.vector.tensor_copy(
                        out=valid(x1[:]),
                        in_=pp[:].rearrange("p (h w) -> p h w", w=W))
                    x1_t.append(x1)

                    fo = fresh(fop, "fo", HW, 8)
                    for j in range(4):
                        slot = s0 + j
                        nc.sync.dma_start(
                            out=fo[32 * j:32 * j + C, :],
                            in_=fown_d[C * slot:C * slot + C, :])
                    x1s = fresh(x1sp, "x1s", GRID, 8)
                    nc.vector.tensor_copy(
                        out=valid(x1s[:]),
                        in_=fo[:].rearrange("p (h w) -> p h w", w=W))
                    x1s_t.append(x1s)

                # ---- conv1: 16-way interleave (4 bundles x 4 rowgroups)
                ps1 = [psp.tile([128, HW], f32, tag="ps", name=f"ps1_{rnd}_{b}")
                       for b in range(4)]
                ps1v = [p[:].rearrange("p (h w) -> p h w", w=W) for p in ps1]
                for h0 in (0, 16):
                    for t in range(9):
                        dy, dx = t // 3, t % 3
                        for b in range(4):
                            for j in range(4):
                                cs = (j + b) % 4
                                nc.tensor.matmul(
                                    out=ps1v[b][32 * cs:32 * cs + 32,
                                                h0:h0 + 16, :],
                                    lhsT=w1pn_t[32 * j:32 * j + 32,
                                                t * 32:t * 32 + 32],
                                    rhs=tap_view(x1_t[b][:], 32 * j, 32,
                                                 dy, dx, h0),
                                    start=(t == 0), stop=False,
                                    tile_position=(32 * j, 32 * cs))
                    for t in range(9):
                        dy, dx = t // 3, t % 3
                        for b in range(4):
                            for j in range(4):
                                cs = (j + b) % 4
                                nc.tensor.matmul(
                                    out=ps1v[b][32 * cs:32 * cs + 32,
                                                h0:h0 + 16, :],
                                    lhsT=w1s_t[32 * j:32 * j + 32,
                                               t * 32:t * 32 + 32],
                                    rhs=tap_view(x1s_t[b][:], 32 * j, 32,
                                                 dy, dx, h0),
                                    start=False, stop=(t == 8),
                                    tile_position=(32 * j, 32 * cs))

                x2_t = []
                for b in range(4):
                    r2a = r2p.tile([128, HW], bf16, tag="r2")
                    nc.scalar.activation(out=r2a[:], in_=ps1[b][:],
                                         func=AF.Relu, scale=-0.9)
                    x2 = fresh(x2p, "x2", GRID, 4)
                    nc.vector.tensor_tensor(
                        out=valid(x2[:]),
                        in0=ps1[b][:].rearrange("p (h w) -> p h w", w=W),
                        in1=r2a[:].rearrange("p (h w) -> p h w", w=W),
                        op=ALU.add)
                    x2_t.append(x2)

                # ---- conv2 (K=32)
                ps2 = [psp.tile([128, HW], f32, tag="ps", name=f"ps2_{rnd}_{b}")
                       for b in range(4)]
                ps2v = [p[:].rearrange("p (h w) -> p h w", w=W) for p in ps2]
                for h0 in (0, 16):
                    for t in range(9):
                        dy, dx = t // 3, t % 3
                        for b in range(4):
                            for q in range(4):
                                cs = (q + b + 1) % 4
                                nc.tensor.matmul(
                                    out=ps2v[b][32 * cs:32 * cs + 32,
                                                h0:h0 + 16, :],
                                    lhsT=w2_t[32 * q:32 * q + 32,
                                              t * 32:t * 32 + 32],
                                    rhs=tap_view(x2_t[b][:], 32 * q, 32,
                                                 dy, dx, h0),
                                    start=(t == 0), stop=(t == 8),
                                    tile_position=(32 * q, 32 * cs))

                x3_t = []
                for b in range(4):
                    r2b = r2p.tile([128, HW], bf16, tag="r2")
                    nc.scalar.activation(out=r2b[:], in_=ps2[b][:],
                                         func=AF.Relu, scale=-0.9)
                    x3 = fresh(x3p, "x3", GRID, 4)
                    nc.vector.tensor_tensor(
                        out=valid(x3[:]),
                        in0=ps2[b][:].rearrange("p (h w) -> p h w", w=W),
                        in1=r2b[:].rearrange("p (h w) -> p h w", w=W),
                        op=ALU.add)
                    x3_t.append(x3)

                # ---- conv3 (K=32, M=16)
                ps3 = [psp.tile([128, HW], f32, tag="ps", name=f"ps3_{rnd}_{b}")
                       for b in range(4)]
                ps3v = [p[:].rearrange("p (h w) -> p h w", w=W) for p in ps3]
                for h0 in (0, 16):
                    for t in range(9):
                        dy, dx = t // 3, t % 3
                        for b in range(4):
                            for q in range(4):
                                cs = (q + b + 2) % 4
                                nc.tensor.matmul(
                                    out=ps3v[b][32 * cs:32 * cs + 16,
                                                h0:h0 + 16, :],
                                    lhsT=w3_t[32 * q:32 * q + 32,
                                              t * 16:t * 16 + 16],
                                    rhs=tap_view(x3_t[b][:], 32 * q, 32,
                                                 dy, dx, h0),
                                    start=(t == 0), stop=(t == 8),
                                    tile_position=(32 * q, 32 * cs))

                for b in range(4):
                    r2c = r2p.tile([128, HW], bf16, tag="r2")
                    nc.scalar.activation(out=r2c[:], in_=ps3[b][:],
                                         func=AF.Relu, scale=-0.9)
                    osb = osbp.tile([128, HW], f32, tag="osb")
                    nc.vector.tensor_tensor(out=osb[:], in0=ps3[b][:],
                                            in1=r2c[:], op=ALU.add)
                    for j in range(4):
                        q1 = (j + b) % 4
                        q2 = (q1 + b + 1) % 4
                        q3 = (q2 + b + 2) % 4
                        slot = 16 * rnd + 4 * b + j
                        nc.sync.dma_start(
                            out=y_d[C * slot:C * slot + C, :],
                            in_=osb[32 * q3:32 * q3 + C, :])
    return nc


def _host_prep(feats, edges, w1, b1, w2, b2, w3, b3):
    import ml_dtypes

    feats = np.ascontiguousarray(np.asarray(feats, dtype=np.float32))
    edges = np.asarray(edges)
    w1 = np.asarray(w1, dtype=np.float32)
    w2 = np.asarray(w2, dtype=np.float32)
    w3 = np.asarray(w3, dtype=np.float32)

    # signed adjacency count matrices: pooled = A_s @ F (bidirectional)
    src = edges[:, 0].astype(np.int64)
    sign = edges[:, 1].astype(np.int64)
    dst = edges[:, 2].astype(np.int64)
    A = np.zeros((2, N, N), np.float32)
    for si, m in ((0, sign > 0), (1, sign < 0)):
        np.add.at(A[si], (dst[m], src[m]), 1.0)
        np.add.at(A[si], (src[m], dst[m]), 1.0)

    tabbf = feats.reshape(N, CHW).astype(ml_dtypes.bfloat16)
    feats2d_bf = tabbf.reshape(N * C, HW)

    # weight tiles (lhsT layout, replicated across the 4 row slots)
    def wtile(w, ci_lo, ci_n, co_n):
        t = np.zeros((128, 9 * co_n), np.float32)
        for rs in range(4):
            for tp in range(9):
                dy, dx = tp // 3, tp % 3
                t[32 * rs:32 * rs + ci_n, tp * co_n:(tp + 1) * co_n] = \
                    w[:, ci_lo:ci_lo + ci_n, dy, dx].T
        return t.astype(ml_dtypes.bfloat16)

    w1pn = wtile(w1, C, 2 * C, 2 * C)
    w1s = wtile(w1, 0, C, 2 * C)
    w2t = wtile(w2, 0, 2 * C, 2 * C)
    w3t = wtile(w3, 0, 2 * C, C)

    in_maps = []
    for k in range(NCORES):
        own = slice(NPC * k, NPC * (k + 1))
        in_maps.append({
            "tabbf": tabbf,
            "apos": np.ascontiguousarray(A[0][own].T).astype(ml_dtypes.bfloat16),
            "aneg": np.ascontiguousarray(A[1][own].T).astype(ml_dtypes.bfloat16),
            "fown": np.ascontiguousarray(feats2d_bf[C * NPC * k:C * NPC * (k + 1)]),
            "w1pn": w1pn, "w1s": w1s, "w2": w2t, "w3": w3t,
        })
    return in_maps


def kernel(feats, edges, w1, b1, w2, b2, w3, b3):
    import os
    from concourse.bass_utils import run_bass_kernel_spmd

    in_maps = _host_prep(feats, edges, w1, b1, w2, b2, w3, b3)
    with_bias = bool(np.any(np.asarray(b1)) or np.any(np.asarray(b2))
                     or np.any(np.asarray(b3)))
    assert not with_bias, "nonzero conv biases not implemented"

    variant = os.environ.get("KERNEL_VARIANT", "full")
    nc = _prog_cache.get(variant)
    if nc is None:
        nc = _build_program(variant)
        _prog_cache[variant] = nc

    trace = bool(os.environ.get("KERNEL_TRACE"))
    res = run_bass_kernel_spmd(nc, in_maps, core_ids=list(range(NCORES)),
                               trace=trace)
    if trace:
        global last_results
        last_results = res

    out = np.empty((N, C, H, W), np.float32)
    for k in range(NCORES):
        yk = res.results[k]["y"]
        out[NPC * k:NPC * (k + 1)] = yk.reshape(NPC, C, H, W)
    return out


# revision 21
# speedup vs baseline: 1.1033x; 1.1033x over previous
"""GNN message passing + 3x conv3x3 + leaky-relu, distributed over 8 trn2 NeuronCores.

Strategy (node-sharded, 128 nodes/core):
- Pooling (pos/neg masked bidirectional scatter-add) is a dense matmul:
  pooled_s[own 128 nodes, C*HW] = sum_k A_s^T[chunk_k, own].T @ F[chunk_k, C*HW]
  with A_s the [N, N] signed-adjacency count matrix (small ints, exact in
  bf16) built on host from the edge list. Full 128x128 PE utilization,
  F (feats, bf16) streamed once from HBM. No edge-dependent program.
- Pooled rows are reshaped into conv layout (4 nodes x (16 pos + 16 neg)
  partitions) by SBUF->SBUF DMAs, then copied into 34x34 zero-padded grids.
- Convs run as 9 shifted-tap matmuls over the padded grid using strided
  access-pattern views (no im2col), bf16 operands, fp32 PSUM accumulation.
  Matmul issue order is tap-outer / (bundle, rowgroup)-inner so all 16
  32x32 tile_position sub-arrays stream concurrently (MATMUL starts are
  pc-monotone; same-position chains serialize, so interleave across
  positions).
- leaky_relu(x) = x + relu(-0.9 x): one ScalarE activation + one VectorE
  tensor_tensor add per bundle, full 128-lane ops.
"""

import numpy as np

N, C, H, W = 1024, 16, 32, 32
NCORES = 8
NPC = N // NCORES            # nodes per core
CONV_ROUNDS = NPC // 16      # 16 nodes per conv round (4 bundles)
HP = WP = H + 2
GRID = HP * WP
HW = H * W
CHW = C * HW                 # 16384
KCH = N // 128               # 8 K-chunks for the pooling matmul
FT = CHW // 512              # 32 free-dim tiles of 512 for the pooling matmul

_prog_cache = {}


def _make_tile_context(nc):
    """TileContext whose lowering splits multi-sem waits onto nop carriers
    (this walrus build accepts at most one sync wait per instruction) and
    whose tail drain does the same."""
    import concourse.mybir as mybir
    import concourse.tile as tile

    class _TC(tile.TileContext):
        def _lower_ordered_insts(self, ordered):
            for bb_name, insts in ordered.items():
                out = []
                for inst in insts:
                    si = inst.sync_info
                    waits = list(si.on_wait) if si is not None and si.on_wait else []
                    if len(waits) > 1:
                        for w in waits[:-1]:
                            car = mybir.InstNoOp(
                                name=self.nc.get_next_instruction_name(),
                                ins=[], outs=[])
                            car.engine = inst.engine
                            car.sync_info = mybir.SyncInfo(on_wait=[w], on_update=[])
                            self.nc.register_instruction(car, overwrite=True)
                            out.append(car)
                        inst.sync_info = mybir.SyncInfo(
                            on_wait=[waits[-1]],
                            on_update=list(si.on_update) if si.on_update else [])
                    out.append(inst)
                insts[:] = out
            return super()._lower_ordered_insts(ordered)

        def _drain_and_barrier(self, tick_clock, wait_clock):
            clock = tick_clock.global_clock
            allocated = wait_clock.sems.allocated()
            for proc, tick in enumerate(clock):
                if tick > 0 and proc in allocated:
                    n = self.nc.sync.nop(nofuse=True, hint="tailwait")
                    n.wait_op(allocated[proc], tick, "sem-ge")
            self.nc.sync.drain()
            self.nc.all_engine_barrier()
            assert self.sems is not None
            popped = self.nc._tile_sem_poison_stack.pop()
            assert popped is self._sem_poison
            self.nc.clear_and_free_semaphores(list(self.sems.allocated().values()))
            self.nc.all_engine_barrier()

    return _TC(nc)


def _build_program(variant="full"):
    import concourse.bass as bass
    import concourse.mybir as mybir

    do_pool = variant in ("full", "pool")
    do_conv = variant in ("full", "conv")

    f32 = mybir.dt.float32
    bf16 = mybir.dt.bfloat16
    AF = mybir.ActivationFunctionType
    ALU = mybir.AluOpType

    nc = bass.Bass()
    tabbf_d = nc.dram_tensor("tabbf", [N, CHW], bf16, kind="ExternalInput")
    apos_d = nc.dram_tensor("apos", [N, NPC], bf16, kind="ExternalInput")
    aneg_d = nc.dram_tensor("aneg", [N, NPC], bf16, kind="ExternalInput")
    # fown2 row 32*slot + c (c<16 real, c>=16 zero) = own feats, conv layout
    fown_d = nc.dram_tensor("fown", [NPC * 32, HW], bf16, kind="ExternalInput")
    w1pn_d = nc.dram_tensor("w1pn", [128, 9 * 32], bf16, kind="ExternalInput")
    w1s_d = nc.dram_tensor("w1s", [128, 9 * 32], bf16, kind="ExternalInput")
    w2_d = nc.dram_tensor("w2", [128, 9 * 32], bf16, kind="ExternalInput")
    w3_d = nc.dram_tensor("w3", [128, 9 * 16], bf16, kind="ExternalInput")
    # y2 row 128*bundle + 32*q3(j,b) + c; host unscrambles
    y_d = nc.dram_tensor("y", [NPC * 32, HW], f32, kind="ExternalOutput")
    # DRAM scratch for pooled: row n = own node, cols = [si, c, hw]
    scr_d = nc.dram_tensor("scr", [NPC, 2 * CHW], bf16, kind="Internal")

    def valid(ap_grid):
        # [p, GRID] tile AP -> [p, 32, 32] interior view of the 34x34 grid
        return ap_grid.rearrange("p (h w) -> p h w", w=WP)[:, 1:H + 1, 1:W + 1]

    def tap_view(ap_grid, base, k, dy, dx, h0):
        # rhs view for tap (dy,dx), output rows [h0, h0+16), K channels at
        # partition `base`
        g3 = ap_grid.rearrange("p (h w) -> p h w", w=WP)
        return g3[base:base + k, h0 + dy:h0 + dy + 16, dx:dx + W]

    tc = _make_tile_context(nc)
    with tc:
        with (tc.tile_pool(name="cw", bufs=1) as cw,
              tc.tile_pool(name="rhsp", bufs=3) as rhsp,
              tc.tile_pool(name="poolbuf", bufs=3) as poolbuf,
              tc.tile_pool(name="ppp", bufs=4) as ppp,
              tc.tile_pool(name="fop", bufs=8) as fop,
              tc.tile_pool(name="x1pnp", bufs=8) as x1pnp,
              tc.tile_pool(name="x1sp", bufs=8) as x1sp,
              tc.tile_pool(name="x2p", bufs=4) as x2p,
              tc.tile_pool(name="x3p", bufs=4) as x3p,
              tc.tile_pool(name="r2p", bufs=4) as r2p,
              tc.tile_pool(name="osbp", bufs=4) as osbp,
              tc.tile_pool(name="psp", bufs=4, space="PSUM") as psp):
            # ---- constant loads
            apos_t = cw.tile([128, N], bf16)
            aneg_t = cw.tile([128, N], bf16)
            nc.sync.dma_start(
                out=apos_t[:].rearrange("p (k m) -> p k m", k=KCH),
                in_=apos_d[:].rearrange("(k p) m -> p k m", p=128))
            nc.sync.dma_start(
                out=aneg_t[:].rearrange("p (k m) -> p k m", k=KCH),
                in_=aneg_d[:].rearrange("(k p) m -> p k m", p=128))
            w1pn_t = cw.tile([128, 9 * 32], bf16)
            nc.sync.dma_start(out=w1pn_t[:], in_=w1pn_d[:])
            w1s_t = cw.tile([128, 9 * 32], bf16)
            nc.sync.dma_start(out=w1s_t[:], in_=w1s_d[:])
            w2_t = cw.tile([128, 9 * 32], bf16)
            nc.sync.dma_start(out=w2_t[:], in_=w2_d[:])
            w3_t = cw.tile([128, 9 * 16], bf16)
            nc.sync.dma_start(out=w3_t[:], in_=w3_d[:])

            # ---- pooling: pooled_s[own n, CHW] = A_s^T.T @ F, bf16->f32 psum
            # Streamed per 512-col free tile to DRAM scratch (row n holds
            # node n's [C, HW] block; scr col = c*HW + hw).
            if do_pool:
                for ft in range(FT):
                    rhs_t = rhsp.tile([128, KCH * 512], bf16, tag="rhs")
                    nc.sync.dma_start(
                        out=rhs_t[:].rearrange("p (k f) -> p k f", k=KCH),
                        in_=tabbf_d[:, 512 * ft:512 * (ft + 1)]
                            .rearrange("(k p) f -> p k f", p=128))
                    ps = psp.tile([128, HW], f32, tag="ps")
                    for k in range(KCH):
                        nc.tensor.matmul(
                            out=ps[:, 0:512],
                            lhsT=apos_t[:, 128 * k:128 * (k + 1)],
                            rhs=rhs_t[:, 512 * k:512 * (k + 1)],
                            start=(k == 0), stop=(k == KCH - 1))
                    for k in range(KCH):
                        nc.tensor.matmul(
                            out=ps[:, 512:1024],
                            lhsT=aneg_t[:, 128 * k:128 * (k + 1)],
                            rhs=rhs_t[:, 512 * k:512 * (k + 1)],
                            start=(k == 0), stop=(k == KCH - 1))
                    stg = poolbuf.tile([128, HW], bf16, tag="stg")
                    nc.vector.tensor_copy(out=stg[:], in_=ps[:])
                    # scr cols: si*CHW + 512*ft + f
                    nc.sync.dma_start(
                        out=scr_d[:].rearrange("n (s cf) -> n s cf", s=2)
                            [:, :, 512 * ft:512 * (ft + 1)],
                        in_=stg[:].rearrange("p (s f) -> p s f", s=2))
            else:
                zstg = poolbuf.tile([128, HW], bf16, tag="zstg")
                nc.vector.memset(zstg[:], 0.0)
                for i in range(2 * CHW // HW):
                    nc.sync.dma_start(
                        out=scr_d[:, HW * i:HW * (i + 1)], in_=zstg[:])

            if not do_conv:
                # park something in y so the output is defined
                zz = osbp.tile([128, HW], f32, tag="osb")
                nc.vector.memset(zz[:], 0.0)
                for bu in range(NPC // 4):
                    nc.sync.dma_start(out=y_d[128 * bu:128 * (bu + 1), :],
                                      in_=zz[:])
                return nc

            memset_count = {}

            def fresh(pool, name, width, bufs):
                t = pool.tile([128, width], bf16, tag=name)
                c = memset_count.get(name, 0)
                if c < bufs:
                    nc.vector.memset(t[:], 0.0)
                    memset_count[name] = c + 1
                return t

            for rnd in range(CONV_ROUNDS):
                # ---- build grids for the 4 bundles of this round
                x1_t, x1s_t = [], []
                for b in range(4):
                    s0 = 16 * rnd + 4 * b
                    pp = ppp.tile([128, HW], bf16, tag="pp")
                    for j in range(4):
                        # scr row cols (si, c, f) -> pp partitions 32j+16si+c
                        nc.sync.dma_start(
                            out=pp[32 * j:32 * j + 32, :],
                            in_=scr_d[s0 + j, :]
                                .rearrange("(c f) -> c f", c=32))
                    x1 = fresh(x1pnp, "x1pn", GRID, 8)
                    nc.vector.tensor_copy(
                        out=valid(x1[:]),
                        in_=pp[:].rearrange("p (h w) -> p h w", w=W))
                    x1_t.append(x1)

                    fo = fop.tile([128, HW], bf16, tag="fo")
                    nc.scalar.dma_start(
                        out=fo[:], in_=fown_d[32 * s0:32 * s0 + 128, :])
                    x1s = fresh(x1sp, "x1s", GRID, 8)
                    nc.vector.tensor_copy(
                        out=valid(x1s[:]),
                        in_=fo[:].rearrange("p (h w) -> p h w", w=W))
                    x1s_t.append(x1s)

                # ---- conv1: 16-way interleave (4 bundles x 4 rowgroups)
                ps1 = [psp.tile([128, HW], f32, tag="ps", name=f"ps1_{rnd}_{b}")
                       for b in range(4)]
                ps1v = [p[:].rearrange("p (h w) -> p h w", w=W) for p in ps1]
                for h0 in (0, 16):
                    for t in range(9):
                        dy, dx = t // 3, t % 3
                        for b in range(4):
                            for j in range(4):
                                cs = (j + b) % 4
                                nc.tensor.matmul(
                                    out=ps1v[b][32 * cs:32 * cs + 32,
                                                h0:h0 + 16, :],
                                    lhsT=w1pn_t[32 * j:32 * j + 32,
                                                t * 32:t * 32 + 32],
                                    rhs=tap_view(x1_t[b][:], 32 * j, 32,
                                                 dy, dx, h0),
                                    start=(t == 0), stop=False,
                                    tile_position=(32 * j, 32 * cs))
                    for t in range(9):
                        dy, dx = t // 3, t % 3
                        for b in range(4):
                            for j in range(4):
                                cs = (j + b) % 4
                                nc.tensor.matmul(
                                    out=ps1v[b][32 * cs:32 * cs + 32,
                                                h0:h0 + 16, :],
                                    lhsT=w1s_t[32 * j:32 * j + 32,
                                               t * 32:t * 32 + 32],
                                    rhs=tap_view(x1s_t[b][:], 32 * j, 32,
                                                 dy, dx, h0),
                                    start=False, stop=(t == 8),
                                    tile_position=(32 * j, 32 * cs))

                x2_t = []
                for b in range(4):
                    r2a = r2p.tile([128, HW], bf16, tag="r2")
                    nc.scalar.activation(out=r2a[:], in_=ps1[b][:],
                                         func=AF.Relu, scale=-0.9)
                    x2 = fresh(x2p, "x2", GRID, 4)
                    nc.vector.tensor_tensor(
                        out=valid(x2[:]),
                        in0=ps1[b][:].rearrange("p (h w) -> p h w", w=W),
                        in1=r2a[:].rearrange("p (h w) -> p h w", w=W),
                        op=ALU.add)
                    x2_t.append(x2)

                # ---- conv2 (K=32)
                ps2 = [psp.tile([128, HW], f32, tag="ps", name=f"ps2_{rnd}_{b}")
                       for b in range(4)]
                ps2v = [p[:].rearrange("p (h w) -> p h w", w=W) for p in ps2]
                for h0 in (0, 16):
                    for t in range(9):
                        dy, dx = t // 3, t % 3
                        for b in range(4):
                            for q in range(4):
                                cs = (q + b + 1) % 4
                                nc.tensor.matmul(
                                    out=ps2v[b][32 * cs:32 * cs + 32,
                                                h0:h0 + 16, :],
                                    lhsT=w2_t[32 * q:32 * q + 32,
                                              t * 32:t * 32 + 32],
                                    rhs=tap_view(x2_t[b][:], 32 * q, 32,
                                                 dy, dx, h0),
                                    start=(t == 0), stop=(t == 8),
                                    tile_position=(32 * q, 32 * cs))

                x3_t = []
                for b in range(4):
                    r2b = r2p.tile([128, HW], bf16, tag="r2")
                    nc.scalar.activation(out=r2b[:], in_=ps2[b][:],
                                         func=AF.Relu, scale=-0.9)
                    x3 = fresh(x3p, "x3", GRID, 4)
                    nc.vector.tensor_tensor(
                        out=valid(x3[:]),
                        in0=ps2[b][:].rearrange("p (h w) -> p h w", w=W),
                        in1=r2b[:].rearrange("p (h w) -> p h w", w=W),
                        op=ALU.add)
                    x3_t.append(x3)

                # ---- conv3 (K=32, M=16)
                ps3 = [psp.tile([128, HW], f32, tag="ps", name=f"ps3_{rnd}_{b}")
                       for b in range(4)]
                ps3v = [p[:].rearrange("p (h w) -> p h w", w=W) for p in ps3]
                for h0 in (0, 16):
                    for t in range(9):
                        dy, dx = t // 3, t % 3
                        for b in range(4):
                            for q in range(4):
                                cs = (q + b + 2) % 4
                                nc.tensor.matmul(
                                    out=ps3v[b][32 * cs:32 * cs + 16,
                                                h0:h0 + 16, :],
                                    lhsT=w3_t[32 * q:32 * q + 32,
                                              t * 16:t * 16 + 16],
                                    rhs=tap_view(x3_t[b][:], 32 * q, 32,
                                                 dy, dx, h0),
                                    start=(t == 0), stop=(t == 8),
                                    tile_position=(32 * q, 32 * cs))

                for b in range(4):
                    r2c = r2p.tile([128, HW], bf16, tag="r2")
                    nc.scalar.activation(out=r2c[:], in_=ps3[b][:],
                                         func=AF.Relu, scale=-0.9)
                    osb = osbp.tile([128, HW], f32, tag="osb")
                    nc.vector.tensor_tensor(out=osb[:], in0=ps3[b][:],
                                            in1=r2c[:], op=ALU.add)
                    bu = 4 * rnd + b
                    nc.scalar.dma_start(
                        out=y_d[128 * bu:128 * (bu + 1), :], in_=osb[:])
    return nc


def _host_prep(feats, edges, w1, b1, w2, b2, w3, b3):
    import ml_dtypes

    feats = np.ascontiguousarray(np.asarray(feats, dtype=np.float32))
    edges = np.asarray(edges)
    w1 = np.asarray(w1, dtype=np.float32)
    w2 = np.asarray(w2, dtype=np.float32)
    w3 = np.asarray(w3, dtype=np.float32)

    # signed adjacency count matrices: pooled = A_s @ F (bidirectional)
    src = edges[:, 0].astype(np.int64)
    sign = edges[:, 1].astype(np.int64)
    dst = edges[:, 2].astype(np.int64)
    A = np.zeros((2, N, N), np.float32)
    for si, m in ((0, sign > 0), (1, sign < 0)):
        np.add.at(A[si], (dst[m], src[m]), 1.0)
        np.add.at(A[si], (src[m], dst[m]), 1.0)

    tabbf = feats.reshape(N, CHW).astype(ml_dtypes.bfloat16)
    feats2d_bf = tabbf.reshape(N * C, HW)
    # fown2: row 32*slot + c = own feats channel c (c<16), zeros above
    fown2 = np.zeros((N, 32, HW), ml_dtypes.bfloat16)
    fown2[:, :C, :] = feats2d_bf.reshape(N, C, HW)
    fown2 = fown2.reshape(N * 32, HW)

    # weight tiles (lhsT layout, replicated across the 4 row slots)
    def wtile(w, ci_lo, ci_n, co_n):
        t = np.zeros((128, 9 * co_n), np.float32)
        for rs in range(4):
            for tp in range(9):
                dy, dx = tp // 3, tp % 3
                t[32 * rs:32 * rs + ci_n, tp * co_n:(tp + 1) * co_n] = \
                    w[:, ci_lo:ci_lo + ci_n, dy, dx].T
        return t.astype(ml_dtypes.bfloat16)

    w1pn = wtile(w1, C, 2 * C, 2 * C)
    w1s = wtile(w1, 0, C, 2 * C)
    w2t = wtile(w2, 0, 2 * C, 2 * C)
    w3t = wtile(w3, 0, 2 * C, C)

    in_maps = []
    for k in range(NCORES):
        own = slice(NPC * k, NPC * (k + 1))
        in_maps.append({
            "tabbf": tabbf,
            "apos": np.ascontiguousarray(A[0][own].T).astype(ml_dtypes.bfloat16),
            "aneg": np.ascontiguousarray(A[1][own].T).astype(ml_dtypes.bfloat16),
            "fown": np.ascontiguousarray(fown2[32 * NPC * k:32 * NPC * (k + 1)]),
            "w1pn": w1pn, "w1s": w1s, "w2": w2t, "w3": w3t,
        })
    return in_maps


def kernel(feats, edges, w1, b1, w2, b2, w3, b3):
    import os
    from concourse.bass_utils import run_bass_kernel_spmd

    in_maps = _host_prep(feats, edges, w1, b1, w2, b2, w3, b3)
    with_bias = bool(np.any(np.asarray(b1)) or np.any(np.asarray(b2))
                     or np.any(np.asarray(b3)))
    assert not with_bias, "nonzero conv biases not implemented"

    variant = os.environ.get("KERNEL_VARIANT", "full")
    nc = _prog_cache.get(variant)
    if nc is None:
        nc = _build_program(variant)
        _prog_cache[variant] = nc

    trace = bool(os.environ.get("KERNEL_TRACE"))
    res = run_bass_kernel_spmd(nc, in_maps, core_ids=list(range(NCORES)),
                               trace=trace)
    if trace:
        global last_results
        last_results = res

    # y2 row 128*bu + 32*q3 + c holds node (4*bu + j)'s channel c, where
    # q3 = perm chain of (j, b) with b = bu % 4
    out = np.empty((N, C, H, W), np.float32)
    q3map = np.empty((4, 4), np.int64)  # [b, j] -> q3
    for b in range(4):
        for j in range(4):
            q1 = (j + b) % 4
            q2 = (q1 + b + 1) % 4
            q3map[b, j] = (q2 + b + 2) % 4
    for k in range(NCORES):
        yk = res.results[k]["y"].reshape(NPC // 4, 4, 32, HW)
        for bu in range(NPC // 4):
            b = bu % 4
            for j in range(4):
                node = NPC * k + 4 * bu + j
                out[node] = yk[bu, q3map[b, j], :C].reshape(C, H, W)
    return out


# revision 25
# speedup vs baseline: 1.2242x; 1.1096x over previous
"""GNN message passing + 3x conv3x3 + leaky-relu, distributed over 8 trn2 NeuronCores.

Strategy (node-sharded, 128 nodes/core):
- Pooling (pos/neg masked bidirectional scatter-add) is a dense matmul:
  pooled_s[own 128 nodes, C*HW] = sum_k A_s^T[chunk_k, own].T @ F[chunk_k, C*HW]
  with A_s the [N, N] signed-adjacency count matrix (small ints, exact in
  bf16) built on host from the edge list. Full 128x128 PE utilization,
  F (feats, bf16) streamed once from HBM. No edge-dependent program.
- Pooled rows are reshaped into conv layout (4 nodes x (16 pos + 16 neg)
  partitions) by SBUF->SBUF DMAs, then copied into 34x34 zero-padded grids.
- Convs run as 9 shifted-tap matmuls over the padded grid using strided
  access-pattern views (no im2col), bf16 operands, fp32 PSUM accumulation.
  Matmul issue order is tap-outer / (bundle, rowgroup)-inner so all 16
  32x32 tile_position sub-arrays stream concurrently (MATMUL starts are
  pc-monotone; same-position chains serialize, so interleave across
  positions).
- leaky_relu(x) = x + relu(-0.9 x): one ScalarE activation + one VectorE
  tensor_tensor add per bundle, full 128-lane ops.
"""

import numpy as np

N, C, H, W = 1024, 16, 32, 32
NCORES = 8
NPC = N // NCORES            # nodes per core
CONV_ROUNDS = NPC // 16      # 16 nodes per conv round (4 bundles)
HP = WP = H + 2
GRID = HP * WP
HW = H * W
CHW = C * HW                 # 16384
KCH = N // 128               # 8 K-chunks for the pooling matmul
FT = CHW // 512              # 32 free-dim tiles of 512 for the pooling matmul

_prog_cache = {}


def _make_tile_context(nc):
    """TileContext whose lowering splits multi-sem waits onto nop carriers
    (this walrus build accepts at most one sync wait per instruction) and
    whose tail drain does the same."""
    import concourse.mybir as mybir
    import concourse.tile as tile

    class _TC(tile.TileContext):
        def _lower_ordered_insts(self, ordered):
            for bb_name, insts in ordered.items():
                out = []
                for inst in insts:
                    si = inst.sync_info
                    waits = list(si.on_wait) if si is not None and si.on_wait else []
                    if len(waits) > 1:
                        for w in waits[:-1]:
                            car = mybir.InstNoOp(
                                name=self.nc.get_next_instruction_name(),
                                ins=[], outs=[])
                            car.engine = inst.engine
                            car.sync_info = mybir.SyncInfo(on_wait=[w], on_update=[])
                            self.nc.register_instruction(car, overwrite=True)
                            out.append(car)
                        inst.sync_info = mybir.SyncInfo(
                            on_wait=[waits[-1]],
                            on_update=list(si.on_update) if si.on_update else [])
                    out.append(inst)
                insts[:] = out
            return super()._lower_ordered_insts(ordered)

        def _drain_and_barrier(self, tick_clock, wait_clock):
            clock = tick_clock.global_clock
            allocated = wait_clock.sems.allocated()
            for proc, tick in enumerate(clock):
                if tick > 0 and proc in allocated:
                    n = self.nc.sync.nop(nofuse=True, hint="tailwait")
                    n.wait_op(allocated[proc], tick, "sem-ge")
            self.nc.sync.drain()
            self.nc.all_engine_barrier()
            assert self.sems is not None
            popped = self.nc._tile_sem_poison_stack.pop()
            assert popped is self._sem_poison
            self.nc.clear_and_free_semaphores(list(self.sems.allocated().values()))
            self.nc.all_engine_barrier()

    return _TC(nc)


def _build_program(variant="full"):
    import concourse.bass as bass
    import concourse.mybir as mybir

    do_pool = variant in ("full", "pool")
    do_conv = variant in ("full", "conv")

    f32 = mybir.dt.float32
    bf16 = mybir.dt.bfloat16
    AF = mybir.ActivationFunctionType
    ALU = mybir.AluOpType

    nc = bass.Bass()
    tabbf_d = nc.dram_tensor("tabbf", [N, CHW], bf16, kind="ExternalInput")
    apos_d = nc.dram_tensor("apos", [N, NPC], bf16, kind="ExternalInput")
    aneg_d = nc.dram_tensor("aneg", [N, NPC], bf16, kind="ExternalInput")
    # fown2 row 32*slot + c (c<16 real, c>=16 zero) = own feats, conv layout
    fown_d = nc.dram_tensor("fown", [NPC * 32, HW], bf16, kind="ExternalInput")
    w1pn_d = nc.dram_tensor("w1pn", [128, 9 * 32], bf16, kind="ExternalInput")
    w1s_d = nc.dram_tensor("w1s", [128, 9 * 32], bf16, kind="ExternalInput")
    w2_d = nc.dram_tensor("w2", [128, 9 * 32], bf16, kind="ExternalInput")
    w3_d = nc.dram_tensor("w3", [128, 9 * 16], bf16, kind="ExternalInput")
    # y2 row 128*bundle + 32*q3(j,b) + c; host unscrambles
    y_d = nc.dram_tensor("y", [NPC * 32, HW], f32, kind="ExternalOutput")
    # DRAM scratch for pooled: row n = own node, cols = [si, c, hw]
    scr_d = nc.dram_tensor("scr", [NPC, 2 * CHW], bf16, kind="Internal")

    def valid(ap_grid):
        # [p, GRID] tile AP -> [p, 32, 32] interior view of the 34x34 grid
        return ap_grid.rearrange("p (h w) -> p h w", w=WP)[:, 1:H + 1, 1:W + 1]

    def tap_view(ap_grid, base, k, dy, dx, h0):
        # rhs view for tap (dy,dx), output rows [h0, h0+16), K channels at
        # partition `base`
        g3 = ap_grid.rearrange("p (h w) -> p h w", w=WP)
        return g3[base:base + k, h0 + dy:h0 + dy + 16, dx:dx + W]

    tc = _make_tile_context(nc)
    with tc:
        with (tc.tile_pool(name="cw", bufs=1) as cw,
              tc.tile_pool(name="rhsp", bufs=3) as rhsp,
              tc.tile_pool(name="poolbuf", bufs=3) as poolbuf,
              tc.tile_pool(name="ppp", bufs=4) as ppp,
              tc.tile_pool(name="fop", bufs=8) as fop,
              tc.tile_pool(name="x1pnp", bufs=8) as x1pnp,
              tc.tile_pool(name="x1sp", bufs=8) as x1sp,
              tc.tile_pool(name="x2p", bufs=4) as x2p,
              tc.tile_pool(name="x3p", bufs=4) as x3p,
              tc.tile_pool(name="osbp", bufs=4) as osbp,
              tc.tile_pool(name="psp", bufs=4, space="PSUM") as psp):
            # ---- constant loads
            apos_t = cw.tile([128, N], bf16)
            aneg_t = cw.tile([128, N], bf16)
            nc.sync.dma_start(
                out=apos_t[:].rearrange("p (k m) -> p k m", k=KCH),
                in_=apos_d[:].rearrange("(k p) m -> p k m", p=128))
            nc.sync.dma_start(
                out=aneg_t[:].rearrange("p (k m) -> p k m", k=KCH),
                in_=aneg_d[:].rearrange("(k p) m -> p k m", p=128))
            w1pn_t = cw.tile([128, 9 * 32], bf16)
            nc.sync.dma_start(out=w1pn_t[:], in_=w1pn_d[:])
            w1s_t = cw.tile([128, 9 * 32], bf16)
            nc.sync.dma_start(out=w1s_t[:], in_=w1s_d[:])
            w2_t = cw.tile([128, 9 * 32], bf16)
            nc.sync.dma_start(out=w2_t[:], in_=w2_d[:])
            w3_t = cw.tile([128, 9 * 16], bf16)
            nc.sync.dma_start(out=w3_t[:], in_=w3_d[:])

            # ---- pooling: pooled_s[own n, CHW] = A_s^T.T @ F, bf16->f32 psum
            # Streamed per 512-col free tile to DRAM scratch (row n holds
            # node n's [C, HW] block; scr col = c*HW + hw).
            if do_pool:
                for ft in range(FT):
                    rhs_t = rhsp.tile([128, KCH * 512], bf16, tag="rhs")
                    nc.sync.dma_start(
                        out=rhs_t[:].rearrange("p (k f) -> p k f", k=KCH),
                        in_=tabbf_d[:, 512 * ft:512 * (ft + 1)]
                            .rearrange("(k p) f -> p k f", p=128))
                    ps = psp.tile([128, HW], f32, tag="ps")
                    for k in range(KCH):
                        nc.tensor.matmul(
                            out=ps[:, 0:512],
                            lhsT=apos_t[:, 128 * k:128 * (k + 1)],
                            rhs=rhs_t[:, 512 * k:512 * (k + 1)],
                            start=(k == 0), stop=(k == KCH - 1))
                    for k in range(KCH):
                        nc.tensor.matmul(
                            out=ps[:, 512:1024],
                            lhsT=aneg_t[:, 128 * k:128 * (k + 1)],
                            rhs=rhs_t[:, 512 * k:512 * (k + 1)],
                            start=(k == 0), stop=(k == KCH - 1))
                    stg = poolbuf.tile([128, HW], bf16, tag="stg")
                    nc.vector.tensor_copy(out=stg[:], in_=ps[:])
                    # scr cols: si*CHW + 512*ft + f
                    nc.sync.dma_start(
                        out=scr_d[:].rearrange("n (s cf) -> n s cf", s=2)
                            [:, :, 512 * ft:512 * (ft + 1)],
                        in_=stg[:].rearrange("p (s f) -> p s f", s=2))
            else:
                zstg = poolbuf.tile([128, HW], bf16, tag="zstg")
                nc.vector.memset(zstg[:], 0.0)
                for i in range(2 * CHW // HW):
                    nc.sync.dma_start(
                        out=scr_d[:, HW * i:HW * (i + 1)], in_=zstg[:])

            if not do_conv:
                # park something in y so the output is defined
                zz = osbp.tile([128, HW], f32, tag="osb")
                nc.vector.memset(zz[:], 0.0)
                for bu in range(NPC // 4):
                    nc.sync.dma_start(out=y_d[128 * bu:128 * (bu + 1), :],
                                      in_=zz[:])
                return nc

            memset_count = {}

            def fresh(pool, name, width, bufs):
                t = pool.tile([128, width], bf16, tag=name)
                c = memset_count.get(name, 0)
                if c < bufs:
                    nc.vector.memset(t[:], 0.0)
                    memset_count[name] = c + 1
                return t

            for rnd in range(CONV_ROUNDS):
                # ---- build grids for the 4 bundles of this round
                x1_t, x1s_t = [], []
                for b in range(4):
                    s0 = 16 * rnd + 4 * b
                    pp = ppp.tile([128, HW], bf16, tag="pp")
                    for j in range(4):
                        # scr row cols (si, c, f) -> pp partitions 32j+16si+c
                        nc.sync.dma_start(
                            out=pp[32 * j:32 * j + 32, :],
                            in_=scr_d[s0 + j, :]
                                .rearrange("(c f) -> c f", c=32))
                    x1 = fresh(x1pnp, "x1pn", GRID, 8)
                    nc.vector.tensor_copy(
                        out=valid(x1[:]),
                        in_=pp[:].rearrange("p (h w) -> p h w", w=W))
                    x1_t.append(x1)

                    fo = fop.tile([128, HW], bf16, tag="fo")
                    nc.scalar.dma_start(
                        out=fo[:], in_=fown_d[32 * s0:32 * s0 + 128, :])
                    x1s = fresh(x1sp, "x1s", GRID, 8)
                    nc.vector.tensor_copy(
                        out=valid(x1s[:]),
                        in_=fo[:].rearrange("p (h w) -> p h w", w=W))
                    x1s_t.append(x1s)

                # ---- conv1: 16-way interleave (4 bundles x 4 rowgroups)
                ps1 = [psp.tile([128, HW], f32, tag="ps", name=f"ps1_{rnd}_{b}")
                       for b in range(4)]
                ps1v = [p[:].rearrange("p (h w) -> p h w", w=W) for p in ps1]
                for h0 in (0, 16):
                    for t in range(9):
                        dy, dx = t // 3, t % 3
                        for b in range(4):
                            for j in range(4):
                                cs = (j + b) % 4
                                nc.tensor.matmul(
                                    out=ps1v[b][32 * cs:32 * cs + 32,
                                                h0:h0 + 16, :],
                                    lhsT=w1pn_t[32 * j:32 * j + 32,
                                                t * 32:t * 32 + 32],
                                    rhs=tap_view(x1_t[b][:], 32 * j, 32,
                                                 dy, dx, h0),
                                    start=(t == 0), stop=False,
                                    tile_position=(32 * j, 32 * cs))
                    for t in range(9):
                        dy, dx = t // 3, t % 3
                        for b in range(4):
                            for j in range(4):
                                cs = (j + b) % 4
                                nc.tensor.matmul(
                                    out=ps1v[b][32 * cs:32 * cs + 32,
                                                h0:h0 + 16, :],
                                    lhsT=w1s_t[32 * j:32 * j + 32,
                                               t * 32:t * 32 + 32],
                                    rhs=tap_view(x1s_t[b][:], 32 * j, 32,
                                                 dy, dx, h0),
                                    start=False, stop=(t == 8),
                                    tile_position=(32 * j, 32 * cs))

                x2_t = []
                for b in range(4):
                    x2 = fresh(x2p, "x2", GRID, 4)
                    nc.scalar.activation(
                        out=valid(x2[:]),
                        in_=ps1[b][:].rearrange("p (h w) -> p h w", w=W),
                        func=AF.Prelu, alpha=0.1)
                    x2_t.append(x2)

                # ---- conv2 (K=32)
                ps2 = [psp.tile([128, HW], f32, tag="ps", name=f"ps2_{rnd}_{b}")
                       for b in range(4)]
                ps2v = [p[:].rearrange("p (h w) -> p h w", w=W) for p in ps2]
                for h0 in (0, 16):
                    for t in range(9):
                        dy, dx = t // 3, t % 3
                        for b in range(4):
                            for q in range(4):
                                cs = (q + b + 1) % 4
                                nc.tensor.matmul(
                                    out=ps2v[b][32 * cs:32 * cs + 32,
                                                h0:h0 + 16, :],
                                    lhsT=w2_t[32 * q:32 * q + 32,
                                              t * 32:t * 32 + 32],
                                    rhs=tap_view(x2_t[b][:], 32 * q, 32,
                                                 dy, dx, h0),
                                    start=(t == 0), stop=(t == 8),
                                    tile_position=(32 * q, 32 * cs))

                x3_t = []
                for b in range(4):
                    x3 = fresh(x3p, "x3", GRID, 4)
                    nc.scalar.activation(
                        out=valid(x3[:]),
                        in_=ps2[b][:].rearrange("p (h w) -> p h w", w=W),
                        func=AF.Prelu, alpha=0.1)
                    x3_t.append(x3)

                # ---- conv3 (K=32, M=16)
                ps3 = [psp.tile([128, HW], f32, tag="ps", name=f"ps3_{rnd}_{b}")
                       for b in range(4)]
                ps3v = [p[:].rearrange("p (h w) -> p h w", w=W) for p in ps3]
                for h0 in (0, 16):
                    for t in range(9):
                        dy, dx = t // 3, t % 3
                        for b in range(4):
                            for q in range(4):
                                cs = (q + b + 2) % 4
                                nc.tensor.matmul(
                                    out=ps3v[b][32 * cs:32 * cs + 16,
                                                h0:h0 + 16, :],
                                    lhsT=w3_t[32 * q:32 * q + 32,
                                              t * 16:t * 16 + 16],
                                    rhs=tap_view(x3_t[b][:], 32 * q, 32,
                                                 dy, dx, h0),
                                    start=(t == 0), stop=(t == 8),
                                    tile_position=(32 * q, 32 * cs))

                for b in range(4):
                    osb = osbp.tile([128, HW], f32, tag="osb")
                    nc.scalar.activation(out=osb[:], in_=ps3[b][:],
                                         func=AF.Prelu, alpha=0.1)
                    bu = 4 * rnd + b
                    nc.scalar.dma_start(
                        out=y_d[128 * bu:128 * (bu + 1), :], in_=osb[:])
    return nc


def _host_prep(feats, edges, w1, b1, w2, b2, w3, b3):
    import ml_dtypes

    feats = np.ascontiguousarray(np.asarray(feats, dtype=np.float32))
    edges = np.asarray(edges)
    w1 = np.asarray(w1, dtype=np.float32)
    w2 = np.asarray(w2, dtype=np.float32)
    w3 = np.asarray(w3, dtype=np.float32)

    # signed adjacency count matrices: pooled = A_s @ F (bidirectional)
    src = edges[:, 0].astype(np.int64)
    sign = edges[:, 1].astype(np.int64)
    dst = edges[:, 2].astype(np.int64)
    A = np.zeros((2, N, N), np.float32)
    for si, m in ((0, sign > 0), (1, sign < 0)):
        np.add.at(A[si], (dst[m], src[m]), 1.0)
        np.add.at(A[si], (src[m], dst[m]), 1.0)

    tabbf = feats.reshape(N, CHW).astype(ml_dtypes.bfloat16)
    feats2d_bf = tabbf.reshape(N * C, HW)
    # fown2: row 32*slot + c = own feats channel c (c<16), zeros above
    fown2 = np.zeros((N, 32, HW), ml_dtypes.bfloat16)
    fown2[:, :C, :] = feats2d_bf.reshape(N, C, HW)
    fown2 = fown2.reshape(N * 32, HW)

    # weight tiles (lhsT layout, replicated across the 4 row slots)
    def wtile(w, ci_lo, ci_n, co_n):
        t = np.zeros((128, 9 * co_n), np.float32)
        for rs in range(4):
            for tp in range(9):
                dy, dx = tp // 3, tp % 3
                t[32 * rs:32 * rs + ci_n, tp * co_n:(tp + 1) * co_n] = \
                    w[:, ci_lo:ci_lo + ci_n, dy, dx].T
        return t.astype(ml_dtypes.bfloat16)

    w1pn = wtile(w1, C, 2 * C, 2 * C)
    w1s = wtile(w1, 0, C, 2 * C)
    w2t = wtile(w2, 0, 2 * C, 2 * C)
    w3t = wtile(w3, 0, 2 * C, C)

    in_maps = []
    for k in range(NCORES):
        own = slice(NPC * k, NPC * (k + 1))
        in_maps.append({
            "tabbf": tabbf,
            "apos": np.ascontiguousarray(A[0][own].T).astype(ml_dtypes.bfloat16),
            "aneg": np.ascontiguousarray(A[1][own].T).astype(ml_dtypes.bfloat16),
            "fown": np.ascontiguousarray(fown2[32 * NPC * k:32 * NPC * (k + 1)]),
            "w1pn": w1pn, "w1s": w1s, "w2": w2t, "w3": w3t,
        })
    return in_maps


def kernel(feats, edges, w1, b1, w2, b2, w3, b3):
    import os
    from concourse.bass_utils import run_bass_kernel_spmd

    in_maps = _host_prep(feats, edges, w1, b1, w2, b2, w3, b3)
    with_bias = bool(np.any(np.asarray(b1)) or np.any(np.asarray(b2))
                     or np.any(np.asarray(b3)))
    assert not with_bias, "nonzero conv biases not implemented"

    variant = os.environ.get("KERNEL_VARIANT", "full")
    nc = _prog_cache.get(variant)
    if nc is None:
        nc = _build_program(variant)
        _prog_cache[variant] = nc

    trace = bool(os.environ.get("KERNEL_TRACE"))
    res = run_bass_kernel_spmd(nc, in_maps, core_ids=list(range(NCORES)),
                               trace=trace)
    if trace:
        global last_results
        last_results = res

    # y2 row 128*bu + 32*q3 + c holds node (4*bu + j)'s channel c, where
    # q3 = perm chain of (j, b) with b = bu % 4
    out = np.empty((N, C, H, W), np.float32)
    q3map = np.empty((4, 4), np.int64)  # [b, j] -> q3
    for b in range(4):
        for j in range(4):
            q1 = (j + b) % 4
            q2 = (q1 + b + 1) % 4
            q3map[b, j] = (q2 + b + 2) % 4
    for k in range(NCORES):
        yk = res.results[k]["y"].reshape(NPC // 4, 4, 32, HW)
        for bu in range(NPC // 4):
            b = bu % 4
            for j in range(4):
                node = NPC * k + 4 * bu + j
                out[node] = yk[bu, q3map[b, j], :C].reshape(C, H, W)
    return out
